# revision 7
# baseline (speedup 1.0000x reference)
"""Causal self-attention (B=4, T=2048, C=1024, 16 heads) on 8 trn2 NeuronCores.

Sharding: tensor-parallel over heads (2 heads/core) for QKV + attention,
then per-batch AllToAll reshards from head-split to token-split for the
output projection.  Output tokens are striped: core c owns, for every
batch b, tokens [b*2048 + c*256, b*2048 + (c+1)*256).  The host gather
interleaves them back.

All matmul operands are bf16 (full-rate PE, fp32 PSUM accumulation);
simulated end-to-end rel err ~3.4e-3 vs the 2e-2 gate.  bf16 enables
XBAR DMA transposes, so the PE runs zero transposes: x tiles are cast
f32->bf16 on DVE, then DMA-transposed (16x128 XBAR tiles) straight
into the channel-major xT layout.  NOTE: XBAR transpose requires a
per-partition-contiguous SBUF destination (silent corruption
otherwise) -- v goes through a contiguous staging tile.

Per-core pipeline (identical program on all cores; only the fed W_qkv
column-slice differs):
  per batch b, per 512-token window w (stage1+stage2 interleaved):
    stage1: xn [tok,1024] f32 tiles DMA'd naturally; DVE-cast to bf16;
            XBAR-transposed to xT [c,tok]; qT,kT [128ch x tok] and v
            [tok-major] from x @ W_qkv_slice (evac casts to bf16).
    stage2: causal attention per head: S^T tiles [kt=128, q=512],
            exp on ACT -> bf16 pt (scores/8 ~ N(0,1), no max
            subtraction), multiplicative 0/1 mask on diagonal blocks
            (DVE), AV accumulation with a ones-column in v so PSUM row
            64 carries softmax denominators; normalize via 1/d =
            exp(-ln d) on ACT + gpsimd partition_broadcast + DVE mul,
            writing bf16 yt.
  after batch b: AllToAll #b (512KB/rank, bf16) -> yT [1024ch, 256tok]
  chunk; proj for chunk b-1 runs under batch b's compute (1-batch
  software pipeline); only A2A #3 + proj #3 are exposed at the tail.

Engine assignment: PE: matmuls only.  ACT: exp, yu/ob evac, ln/exp
reciprocal.  DVE: casts, evacuations, diag masks, normalize mul,
memset.  GpSimd: partition_broadcast + collective trigger only.
DMA: x loads, XBAR transposes, y/out movement.
"""

import os
import numpy as np

from concourse import bass, bacc, mybir, tile
from concourse.bass_utils import run_bass_kernel_spmd

F32 = mybir.dt.float32
BF16 = mybir.dt.bfloat16

B, T, C = 4, 2048, 1024
H, D = 16, 64
NCORES = 8
HPC = H // NCORES            # heads per core = 2
QKC = HPC * D                # per-core q/k/v channels = 128
BT = B * T                   # 8192 tokens total
TPS = BT // NCORES           # tokens per core after A2A = 1024
CHK = TPS // B               # tokens per (core, batch) chunk = 256
P = 128
TW = 512                     # q window
NW = T // TW                 # windows per batch = 4
NKT = T // P                 # kt tiles per batch = 16
KC = C // P                  # contraction chunks = 8

DIAG_FIRST = os.environ.get("KDIAG", "1") == "1"


def _causal_mask_01() -> np.ndarray:
    """mask[p, m, f] = 1.0 iff kt_local = 128*m + p <= f, for q windows of 512."""
    m = np.zeros((P, NW, TW), dtype=np.float32)
    p = np.arange(P)[:, None, None]
    mm = np.arange(NW)[None, :, None]
    f = np.arange(TW)[None, None, :]
    m[(P * mm + p) <= f] = 1.0
    return m


def build() -> bass.Bass:
    nc = bacc.Bacc(num_devices=NCORES, target_bir_lowering=False)

    x_d = nc.dram_tensor("x", [BT, C], F32, kind="ExternalInput")
    wqkv_d = nc.dram_tensor("wqkv", [C, 3 * QKC], F32, kind="ExternalInput")
    wproj_d = nc.dram_tensor("wproj", [C, C], F32, kind="ExternalInput")
    out_d = nc.dram_tensor("out", [TPS, C], F32, kind="ExternalOutput")

    mask_d = nc.inline_tensor(_causal_mask_01(), name="mask01")

    with tile.TileContext(nc) as tc:
        from contextlib import ExitStack

        with ExitStack() as ctx:
            # ---- persistent pools ----
            wq_pool = ctx.enter_context(tc.tile_pool(name="wq", bufs=1))
            dram = ctx.enter_context(tc.tile_pool(name="dram", bufs=1, space="DRAM"))

            wqkv_sb = wq_pool.tile([P, KC, 3 * QKC], BF16, tag="wqkv")
            mask_sb = wq_pool.tile([P, NW, TW], BF16, tag="mask")
            wproj_sb = wq_pool.tile([P, KC, C], BF16, tag="wproj")
            with tc.tile_pool(name="stage", bufs=2) as stg:
                wqkv_st = stg.tile([P, KC, 3 * QKC], F32, tag="st3", bufs=1)
                nc.sync.dma_start(
                    out=wqkv_st[:],
                    in_=wqkv_d[:, :].rearrange("(k p) n -> p k n", p=P),
                )
                nc.vector.tensor_copy(wqkv_sb[:], wqkv_st[:])
                mask_st = stg.tile([P, NW, TW], F32, tag="stm", bufs=1)
                nc.sync.dma_start(out=mask_st[:], in_=mask_d[:, :, :])
                nc.vector.tensor_copy(mask_sb[:], mask_st[:])
                for kc in range(KC):
                    wproj_st = stg.tile([P, C], F32, tag="wst", name="wproj_st")
                    nc.sync.dma_start(
                        out=wproj_st[:], in_=wproj_d[kc * P : (kc + 1) * P, :]
                    )
                    nc.vector.tensor_copy(wproj_sb[:, kc, :], wproj_st[:])

            y_send = [
                dram.tile([NCORES, QKC, CHK], BF16, tag=f"ys{b}", name=f"y_send{b}")
                for b in range(B)
            ]
            y_recv = [
                dram.tile([NCORES, QKC, CHK], BF16, tag=f"yr{b}", name=f"y_recv{b}")
                for b in range(B)
            ]

            # ---- working pools ----
            xT_pool = ctx.enter_context(tc.tile_pool(name="xT", bufs=2))
            qkv_pool = ctx.enter_context(tc.tile_pool(name="qkv", bufs=2))
            ps1 = ctx.enter_context(tc.tile_pool(name="ps1", bufs=2, space="PSUM"))
            pss = ctx.enter_context(tc.tile_pool(name="pss", bufs=2, space="PSUM"))
            psy = ctx.enter_context(tc.tile_pool(name="psy", bufs=2, space="PSUM"))
            pso = ctx.enter_context(tc.tile_pool(name="pso", bufs=2, space="PSUM"))
            pt_pool = ctx.enter_context(tc.tile_pool(name="pt", bufs=6))
            nrm_pool = ctx.enter_context(tc.tile_pool(name="nrm", bufs=2))
            yt_pool = ctx.enter_context(tc.tile_pool(name="yt", bufs=2))
            yr_pool = ctx.enter_context(tc.tile_pool(name="yr", bufs=2))
            ob_pool = ctx.enter_context(tc.tile_pool(name="ob", bufs=2))

            def stage3(b):
                yr = yr_pool.tile([P, NCORES, CHK], BF16, tag="yr")
                nc.sync.dma_start(
                    out=yr[:], in_=y_recv[b][:, :, :].rearrange("k p t -> p k t")
                )
                for sub in range(CHK // P):
                    for half in range(C // TW):
                        ps_o = pso.tile([P, TW], F32, tag="ps_o")
                        for kc in range(KC):
                            nc.tensor.matmul(
                                ps_o[:],
                                lhsT=yr[:, kc, sub * P : (sub + 1) * P],
                                rhs=wproj_sb[:, kc, half * TW : (half + 1) * TW],
                                start=(kc == 0),
                                stop=(kc == KC - 1),
                            )
                        ob = ob_pool.tile([P, TW], F32, tag="ob")
                        nc.scalar.copy(ob[:], ps_o[:])
                        nc.sync.dma_start(
                            out=out_d[
                                b * CHK + sub * P : b * CHK + (sub + 1) * P,
                                half * TW : (half + 1) * TW,
                            ],
                            in_=ob[:],
                        )

            for b in range(B):
                qT_b = qkv_pool.tile([P, T], BF16, tag="qT")
                kT_b = qkv_pool.tile([P, T], BF16, tag="kT")
                v_b = qkv_pool.tile([P, NKT, HPC, D + 1], BF16, tag="v")
                # ones column for softmax denominators
                nc.vector.memset(v_b[:, :, :, D : D + 1], 1.0)

                for w in range(NW):
                    t0 = b * T + w * TW
                    # ---- stage 1: qT, kT, v for window w ----
                    xT = xT_pool.tile([P, KC, TW], BF16, tag="xT")
                    for s in range(TW // P):
                        xn = xT_pool.tile([P, C], F32, tag="xn", name="xn", bufs=6)
                        nc.sync.dma_start(
                            out=xn[:], in_=x_d[t0 + s * P : t0 + (s + 1) * P, :]
                        )
                        xnb = xT_pool.tile([P, C], BF16, tag="xnb", name="xnb", bufs=6)
                        nc.vector.tensor_copy(xnb[:], xn[:])
                        for kc in range(KC):
                            # XBAR transpose: [128 tok, 128 c] -> [128 c, 128 tok]
                            nc.sync.dma_start_transpose(
                                out=xT[:, kc, s * P : (s + 1) * P],
                                in_=xnb[:, kc * P : (kc + 1) * P],
                            )
                    for which, dst in ((0, qT_b), (1, kT_b)):
                        ps = ps1.tile([P, TW], F32, tag="ps1")
                        for kc in range(KC):
                            nc.tensor.matmul(
                                ps[:],
                                lhsT=wqkv_sb[:, kc, which * QKC : (which + 1) * QKC],
                                rhs=xT[:, kc, :],
                                start=(kc == 0),
                                stop=(kc == KC - 1),
                            )
                        nc.vector.tensor_copy(dst[:, w * TW : (w + 1) * TW], ps[:])
                    ps_vT = ps1.tile([P, TW], F32, tag="ps1", name="ps_vT")
                    for kc in range(KC):
                        nc.tensor.matmul(
                            ps_vT[:],
                            lhsT=wqkv_sb[:, kc, 2 * QKC : 3 * QKC],
                            rhs=xT[:, kc, :],
                            start=(kc == 0),
                            stop=(kc == KC - 1),
                        )
                    vT_sb = xT_pool.tile([P, TW], BF16, tag="vT", name="vT_sb")
                    nc.vector.tensor_copy(vT_sb[:], ps_vT[:])
                    # XBAR-transpose vT into a contiguous staging tile, then one
                    # strided DVE copy into v_b (XBAR needs contiguous dst)
                    vst = xT_pool.tile([P, TW], BF16, tag="vst", name="vst")
                    for s in range(TW // P):
                        nc.sync.dma_start_transpose(
                            out=vst[:, s * P : (s + 1) * P],
                            in_=vT_sb[:, s * P : (s + 1) * P],
                        )
                    jt0 = w * (TW // P)
                    nc.vector.tensor_copy(
                        v_b[:, jt0 : jt0 + TW // P, :, 0:D],
                        vst[:].rearrange("p (s h d) -> p s h d", s=TW // P, h=HPC),
                    )

                    # ---- stage 2: attention for window w ----
                    for h in range(HPC):
                        qT_h = qT_b[h * D : (h + 1) * D, :]
                        kT_h = kT_b[h * D : (h + 1) * D, :]
                        nkt = (w + 1) * (TW // P)
                        ps_y = psy.tile([D + 1, TW], F32, tag="ps_y")
                        jks = list(range(nkt))
                        if DIAG_FIRST:
                            jks = jks[w * (TW // P) :] + jks[: w * (TW // P)]
                        for ji, jk in enumerate(jks):
                            ps_s = pss.tile([P, TW], F32, tag="ps_s")
                            nc.tensor.matmul(
                                ps_s[:],
                                lhsT=kT_h[:, jk * P : (jk + 1) * P],
                                rhs=qT_h[:, w * TW : (w + 1) * TW],
                                start=True,
                                stop=True,
                            )
                            pt = pt_pool.tile([P, TW], BF16, tag="pt")
                            nc.scalar.activation(
                                pt[:],
                                ps_s[:],
                                mybir.ActivationFunctionType.Exp,
                                scale=1.0 / np.sqrt(D),
                            )
                            m = jk - w * (TW // P)
                            if m >= 0:
                                nc.vector.tensor_mul(pt[:], pt[:], mask_sb[:, m, :])
                            nc.tensor.matmul(
                                ps_y[:],
                                lhsT=v_b[:, jk, h, :],
                                rhs=pt[:],
                                start=(ji == 0),
                                stop=(ji == nkt - 1),
                            )
                        yu = yt_pool.tile([D + 1, TW], F32, tag="yu", bufs=4)
                        nc.scalar.copy(yu[:], ps_y[:])
                        # 1/d = exp(-ln d) on ACT (vector.reciprocal is ~3.3us)
                        lnd = nrm_pool.tile([1, TW], F32, tag="lnd")
                        nc.scalar.activation(
                            lnd[:], yu[D : D + 1, :], mybir.ActivationFunctionType.Ln
                        )
                        recip = nrm_pool.tile([1, TW], F32, tag="recip")
                        nc.scalar.activation(
                            recip[:],
                            lnd[:],
                            mybir.ActivationFunctionType.Exp,
                            scale=-1.0,
                        )
                        bc = nrm_pool.tile([D, TW], F32, tag="bc")
                        nc.gpsimd.partition_broadcast(bc[:], recip[:])
                        yt = yt_pool.tile([D, TW], BF16, tag="yt")
                        nc.vector.tensor_mul(yt[:], yu[0:D, :], bc[:])
                        for half in range(TW // CHK):
                            s = (TW // CHK) * w + half
                            nc.sync.dma_start(
                                out=y_send[b][s, h * D : (h + 1) * D, :],
                                in_=yt[:, half * CHK : (half + 1) * CHK],
                            )

                # ---- A2A for batch b; proj for batch b-1 under batch b+1 ----
                nc.gpsimd.collective_compute(
                    "AllToAll",
                    mybir.AluOpType.bypass,
                    replica_groups=[list(range(NCORES))],
                    ins=[y_send[b].opt()],
                    outs=[y_recv[b].opt()],
                )
                if b > 0:
                    stage3(b - 1)
            stage3(B - 1)

    nc.finalize()
    return nc


_NC_CACHE: dict = {}


def _get_nc() -> bass.Bass:
    if "nc" not in _NC_CACHE:
        _NC_CACHE["nc"] = build()
    return _NC_CACHE["nc"]


def shard_inputs(x, W_qkv, W_proj):
    x = np.ascontiguousarray(np.asarray(x, dtype=np.float32).reshape(BT, C))
    W_qkv = np.asarray(W_qkv, dtype=np.float32)
    W_proj = np.ascontiguousarray(np.asarray(W_proj, dtype=np.float32))
    in_maps = []
    for c in range(NCORES):
        cols = slice(QKC * c, QKC * (c + 1))
        w_c = np.ascontiguousarray(
            np.concatenate(
                [W_qkv[:, cols], W_qkv[:, C:][:, cols], W_qkv[:, 2 * C :][:, cols]],
                axis=1,
            )
        )
        in_maps.append({"x": x, "wqkv": w_c, "wproj": W_proj})
    return in_maps


def run(in_maps, trace=False, **kwargs):
    return run_bass_kernel_spmd(
        _get_nc(), in_maps, core_ids=list(range(NCORES)), trace=trace, **kwargs
    )


def gather(res) -> np.ndarray:
    """Un-stripe: core c's out rows are [b*CHK, (b+1)*CHK) = batch b tokens
    [b*T + c*CHK, b*T + (c+1)*CHK)."""
    outs = np.stack([res.results[c]["out"] for c in range(NCORES)])  # [8, TPS, C]
    full = outs.reshape(NCORES, B, CHK, C).transpose(1, 0, 2, 3).reshape(B, T, C)
    return np.ascontiguousarray(full)


def kernel(x, W_qkv, W_proj):
    res = run(shard_inputs(x, W_qkv, W_proj), trace=False)
    return gather(res).astype(np.float32)


# revision 10
# speedup vs baseline: 1.7937x; 1.7937x over previous
"""Causal self-attention (B=4, T=2048, C=1024, 16 heads) on 8 trn2 NeuronCores.

Sharding: tensor-parallel over heads (2 heads/core) for QKV + attention.
The x-transpose (token-major -> channel-major), identical work on every
core, is itself sharded: each core PE-transposes only its 1/8 of the
tokens (fed as the separate `xown` input; cores never load full x) and
a per-batch AllGather shares the bf16 xT through DRAM.  After
attention, a per-batch AllToAll reshards from head-split to
token-split for the output projection.  Output tokens are striped:
core c owns, for every batch b, tokens [b*2048+c*256, b*2048+(c+1)*256);
the host gather interleaves them back.

All matmul operands are bf16 (full-rate PE, fp32 PSUM accumulation);
simulated end-to-end rel err ~3.4e-3 vs the 2e-2 gate.

Per-core pipeline (identical program on all cores; only the fed W_qkv
column-slice and xown token-slice differ):
  upfront: per batch, transpose own 256 tokens (16 PE transposes) and
  trigger AllGather #b -> xT_full[b] [8 src x 1024ch x 256tok] DRAM.
  per batch b, per 512-token window w:
    stage1: DMA xT window [128,8kc,512] from xT_full[b]; qT,kT
            [128ch x tok] and v [tok-major, PE-transposed] from
            x @ W_qkv_slice (evacuations cast PSUM f32 -> bf16).
    stage2: causal attention per head: S^T tiles [kt=128, q=512],
            exp on ACT -> bf16 pt (scores/8 ~ N(0,1), no max
            subtraction), 0/1 mask on diagonal blocks (DVE), AV
            accumulation with a ones-column in v so PSUM row 64
            carries softmax denominators; normalize via 1/d =
            exp(-ln d) on ACT + gpsimd partition_broadcast + DVE mul.
  after batch b: AllToAll #b (512KB/rank bf16); proj for batch b-1
  runs under batch b's compute; only A2A #3 + proj #3 expose a tail.

Engine assignment: PE: matmuls + shard-transposes.  ACT: exp, yu/ob
evac, ln/exp reciprocal.  DVE: casts, evacuations, diag masks,
normalize mul.  GpSimd: partition_broadcast + collective triggers.
"""

import os
import numpy as np

from concourse import bass, bacc, mybir, tile
from concourse.bass_utils import run_bass_kernel_spmd

F32 = mybir.dt.float32
BF16 = mybir.dt.bfloat16

B, T, C = 4, 2048, 1024
H, D = 16, 64
NCORES = 8
HPC = H // NCORES            # heads per core = 2
QKC = HPC * D                # per-core q/k/v channels = 128
BT = B * T                   # 8192 tokens total
TPS = BT // NCORES           # tokens per core after A2A = 1024
CHK = TPS // B               # tokens per (core, batch) chunk = 256
P = 128
TW = 512                     # q window
NW = T // TW                 # windows per batch = 4
NKT = T // P                 # kt tiles per batch = 16
KC = C // P                  # contraction chunks = 8

DIAG_FIRST = os.environ.get("KDIAG", "1") == "1"


def _causal_mask_01() -> np.ndarray:
    """mask[p, m, f] = 1.0 iff kt_local = 128*m + p <= f, for q windows of 512."""
    m = np.zeros((P, NW, TW), dtype=np.float32)
    p = np.arange(P)[:, None, None]
    mm = np.arange(NW)[None, :, None]
    f = np.arange(TW)[None, None, :]
    m[(P * mm + p) <= f] = 1.0
    return m


def build() -> bass.Bass:
    nc = bacc.Bacc(num_devices=NCORES, target_bir_lowering=False)

    xown_d = nc.dram_tensor("xown", [B * CHK, C], F32, kind="ExternalInput")
    wqkv_d = nc.dram_tensor("wqkv", [C, 3 * QKC], F32, kind="ExternalInput")
    wproj_d = nc.dram_tensor("wproj", [C, C], F32, kind="ExternalInput")
    out_d = nc.dram_tensor("out", [TPS, C], F32, kind="ExternalOutput")

    mask_d = nc.inline_tensor(_causal_mask_01(), name="mask01")
    ident_d = nc.inline_tensor(np.eye(P, dtype=np.float32), name="ident")

    with tile.TileContext(nc) as tc:
        from contextlib import ExitStack

        with ExitStack() as ctx:
            # ---- persistent pools ----
            wq_pool = ctx.enter_context(tc.tile_pool(name="wq", bufs=1))
            dram = ctx.enter_context(tc.tile_pool(name="dram", bufs=1, space="DRAM"))

            wqkv_sb = wq_pool.tile([P, KC, 3 * QKC], BF16, tag="wqkv")
            mask_sb = wq_pool.tile([P, NW, TW], BF16, tag="mask")
            identb = wq_pool.tile([P, P], BF16, tag="identb")
            wproj_sb = wq_pool.tile([P, KC, C], BF16, tag="wproj")
            with tc.tile_pool(name="stage", bufs=2) as stg:
                idst = stg.tile([P, P], F32, tag="idst", bufs=1)
                nc.sync.dma_start(out=idst[:], in_=ident_d[:, :])
                nc.vector.tensor_copy(identb[:], idst[:])
                wqkv_st = stg.tile([P, KC, 3 * QKC], F32, tag="st3", bufs=1)
                nc.sync.dma_start(
                    out=wqkv_st[:],
                    in_=wqkv_d[:, :].rearrange("(k p) n -> p k n", p=P),
                )
                nc.vector.tensor_copy(wqkv_sb[:], wqkv_st[:])
                mask_st = stg.tile([P, NW, TW], F32, tag="stm", bufs=1)
                nc.sync.dma_start(out=mask_st[:], in_=mask_d[:, :, :])
                nc.vector.tensor_copy(mask_sb[:], mask_st[:])
                for kc in range(KC):
                    wproj_st = stg.tile([P, C], F32, tag="wst", name="wproj_st")
                    nc.sync.dma_start(
                        out=wproj_st[:], in_=wproj_d[kc * P : (kc + 1) * P, :]
                    )
                    nc.vector.tensor_copy(wproj_sb[:, kc, :], wproj_st[:])

            y_send = [
                dram.tile([NCORES, QKC, CHK], BF16, tag=f"ys{b}", name=f"y_send{b}")
                for b in range(B)
            ]
            y_recv = [
                dram.tile([NCORES, QKC, CHK], BF16, tag=f"yr{b}", name=f"y_recv{b}")
                for b in range(B)
            ]
            xpart = [
                dram.tile([C, CHK], BF16, tag=f"xp{b}", name=f"xpart{b}")
                for b in range(B)
            ]
            xT_full = [
                dram.tile([NCORES, C, CHK], BF16, tag=f"xf{b}", name=f"xT_full{b}")
                for b in range(B)
            ]

            # ---- working pools ----
            xT_pool = ctx.enter_context(tc.tile_pool(name="xT", bufs=2))
            qkv_pool = ctx.enter_context(tc.tile_pool(name="qkv", bufs=2))
            ps1 = ctx.enter_context(tc.tile_pool(name="ps1", bufs=2, space="PSUM"))
            pss = ctx.enter_context(tc.tile_pool(name="pss", bufs=2, space="PSUM"))
            psy = ctx.enter_context(tc.tile_pool(name="psy", bufs=2, space="PSUM"))
            pso = ctx.enter_context(tc.tile_pool(name="pso", bufs=2, space="PSUM"))
            pt_pool = ctx.enter_context(tc.tile_pool(name="pt", bufs=6))
            nrm_pool = ctx.enter_context(tc.tile_pool(name="nrm", bufs=2))
            yt_pool = ctx.enter_context(tc.tile_pool(name="yt", bufs=2))
            yr_pool = ctx.enter_context(tc.tile_pool(name="yr", bufs=2))
            ob_pool = ctx.enter_context(tc.tile_pool(name="ob", bufs=2))

            def allgather(b):
                nc.gpsimd.collective_compute(
                    "AllGather",
                    mybir.AluOpType.bypass,
                    replica_groups=[list(range(NCORES))],
                    ins=[xpart[b].opt()],
                    outs=[xT_full[b].opt()],
                )

            # ---- upfront: transpose own 256-token slice of each batch ----
            for b in range(B):
                xTp = xT_pool.tile([P, KC, CHK], BF16, tag="xTp", name="xTp")
                for s in range(CHK // P):
                    xn = xT_pool.tile([P, C], F32, tag="xn", name="xn", bufs=4)
                    nc.sync.dma_start(
                        out=xn[:],
                        in_=xown_d[b * CHK + s * P : b * CHK + (s + 1) * P, :],
                    )
                    xnb = xT_pool.tile([P, C], BF16, tag="xnb", name="xnb", bufs=4)
                    nc.vector.tensor_copy(xnb[:], xn[:])
                    for kcp in range(KC // 2):
                        ps_t = ps1.tile([P, 2, P], BF16, tag="ps1", name="ps_t")
                        for k2 in range(2):
                            kc = 2 * kcp + k2
                            nc.tensor.transpose(
                                ps_t[:, k2, :],
                                xnb[:, kc * P : (kc + 1) * P],
                                identb[:],
                            )
                        nc.vector.tensor_copy(
                            xTp[:, 2 * kcp : 2 * kcp + 2, s * P : (s + 1) * P],
                            ps_t[:],
                        )
                nc.sync.dma_start(
                    out=xpart[b][:, :].rearrange("(k p) t -> p k t", p=P),
                    in_=xTp[:],
                )
                # interleave AG triggers below; AG0/AG1 fire immediately
                if b <= 1:
                    allgather(b)

            def stage3(b):
                yr = yr_pool.tile([P, NCORES, CHK], BF16, tag="yr")
                nc.sync.dma_start(
                    out=yr[:], in_=y_recv[b][:, :, :].rearrange("k p t -> p k t")
                )
                for sub in range(CHK // P):
                    for half in range(C // TW):
                        ps_o = pso.tile([P, TW], F32, tag="ps_o")
                        for kc in range(KC):
                            nc.tensor.matmul(
                                ps_o[:],
                                lhsT=yr[:, kc, sub * P : (sub + 1) * P],
                                rhs=wproj_sb[:, kc, half * TW : (half + 1) * TW],
                                start=(kc == 0),
                                stop=(kc == KC - 1),
                            )
                        ob = ob_pool.tile([P, TW], F32, tag="ob")
                        nc.scalar.copy(ob[:], ps_o[:])
                        nc.sync.dma_start(
                            out=out_d[
                                b * CHK + sub * P : b * CHK + (sub + 1) * P,
                                half * TW : (half + 1) * TW,
                            ],
                            in_=ob[:],
                        )

            for b in range(B):
                qT_b = qkv_pool.tile([P, T], BF16, tag="qT")
                kT_b = qkv_pool.tile([P, T], BF16, tag="kT")
                v_b = qkv_pool.tile([P, NKT, HPC, D + 1], BF16, tag="v")
                # ones column for softmax denominators
                nc.vector.memset(v_b[:, :, :, D : D + 1], 1.0)

                for w in range(NW):
                    # ---- stage 1: qT, kT, v for window w ----
                    xT = xT_pool.tile([P, KC, TW], BF16, tag="xT")
                    for s in range(TW // CHK):
                        nc.sync.dma_start(
                            out=xT[:, :, s * CHK : (s + 1) * CHK],
                            in_=xT_full[b][2 * w + s, :, :].rearrange(
                                "(k p) t -> p k t", p=P
                            ),
                        )
                    for which, dst in ((0, qT_b), (1, kT_b)):
                        ps = ps1.tile([P, TW], F32, tag="ps1")
                        for kc in range(KC):
                            nc.tensor.matmul(
                                ps[:],
                                lhsT=wqkv_sb[:, kc, which * QKC : (which + 1) * QKC],
                                rhs=xT[:, kc, :],
                                start=(kc == 0),
                                stop=(kc == KC - 1),
                            )
                        nc.vector.tensor_copy(dst[:, w * TW : (w + 1) * TW], ps[:])
                    ps_vT = ps1.tile([P, TW], F32, tag="ps1", name="ps_vT")
                    for kc in range(KC):
                        nc.tensor.matmul(
                            ps_vT[:],
                            lhsT=wqkv_sb[:, kc, 2 * QKC : 3 * QKC],
                            rhs=xT[:, kc, :],
                            start=(kc == 0),
                            stop=(kc == KC - 1),
                        )
                    vT_sb = xT_pool.tile([P, TW], BF16, tag="vT", name="vT_sb")
                    nc.vector.tensor_copy(vT_sb[:], ps_vT[:])
                    ps_v = ps1.tile([P, TW], BF16, tag="ps1", name="ps_v")
                    for s in range(TW // P):
                        nc.tensor.transpose(
                            ps_v[:, s * P : (s + 1) * P],
                            vT_sb[:, s * P : (s + 1) * P],
                            identb[:],
                        )
                    jt0 = w * (TW // P)
                    nc.vector.tensor_copy(
                        v_b[:, jt0 : jt0 + TW // P, :, 0:D],
                        ps_v[:].rearrange("p (s h d) -> p s h d", s=TW // P, h=HPC),
                    )

                    # ---- stage 2: attention for window w ----
                    for h in range(HPC):
                        qT_h = qT_b[h * D : (h + 1) * D, :]
                        kT_h = kT_b[h * D : (h + 1) * D, :]
                        nkt = (w + 1) * (TW // P)
                        ps_y = psy.tile([D + 1, TW], F32, tag="ps_y")
                        jks = list(range(nkt))
                        if DIAG_FIRST:
                            jks = jks[w * (TW // P) :] + jks[: w * (TW // P)]
                        for ji, jk in enumerate(jks):
                            ps_s = pss.tile([P, TW], F32, tag="ps_s")
                            nc.tensor.matmul(
                                ps_s[:],
                                lhsT=kT_h[:, jk * P : (jk + 1) * P],
                                rhs=qT_h[:, w * TW : (w + 1) * TW],
                                start=True,
                                stop=True,
                            )
                            pt = pt_pool.tile([P, TW], BF16, tag="pt")
                            nc.scalar.activation(
                                pt[:],
                                ps_s[:],
                                mybir.ActivationFunctionType.Exp,
                                scale=1.0 / np.sqrt(D),
                            )
                            m = jk - w * (TW // P)
                            if m >= 0:
                                nc.vector.tensor_mul(pt[:], pt[:], mask_sb[:, m, :])
                            nc.tensor.matmul(
                                ps_y[:],
                                lhsT=v_b[:, jk, h, :],
                                rhs=pt[:],
                                start=(ji == 0),
                                stop=(ji == nkt - 1),
                            )
                        yu = yt_pool.tile([D + 1, TW], F32, tag="yu", bufs=4)
                        nc.scalar.copy(yu[:], ps_y[:])
                        # 1/d = exp(-ln d) on ACT (vector.reciprocal is ~3.3us)
                        lnd = nrm_pool.tile([1, TW], F32, tag="lnd")
                        nc.scalar.activation(
                            lnd[:], yu[D : D + 1, :], mybir.ActivationFunctionType.Ln
                        )
                        recip = nrm_pool.tile([1, TW], F32, tag="recip")
                        nc.scalar.activation(
                            recip[:],
                            lnd[:],
                            mybir.ActivationFunctionType.Exp,
                            scale=-1.0,
                        )
                        bc = nrm_pool.tile([D, TW], F32, tag="bc")
                        nc.gpsimd.partition_broadcast(bc[:], recip[:])
                        yt = yt_pool.tile([D, TW], BF16, tag="yt")
                        nc.vector.tensor_mul(yt[:], yu[0:D, :], bc[:])
                        for half in range(TW // CHK):
                            s = (TW // CHK) * w + half
                            nc.sync.dma_start(
                                out=y_send[b][s, h * D : (h + 1) * D, :],
                                in_=yt[:, half * CHK : (half + 1) * CHK],
                            )

                # ---- A2A for batch b; AG b+2; proj for batch b-1 ----
                nc.gpsimd.collective_compute(
                    "AllToAll",
                    mybir.AluOpType.bypass,
                    replica_groups=[list(range(NCORES))],
                    ins=[y_send[b].opt()],
                    outs=[y_recv[b].opt()],
                )
                if b + 2 < B:
                    allgather(b + 2)
                if b > 0:
                    stage3(b - 1)
            stage3(B - 1)

    nc.finalize()
    return nc


_NC_CACHE: dict = {}


def _get_nc() -> bass.Bass:
    if "nc" not in _NC_CACHE:
        _NC_CACHE["nc"] = build()
    return _NC_CACHE["nc"]


def shard_inputs(x, W_qkv, W_proj):
    x = np.ascontiguousarray(np.asarray(x, dtype=np.float32).reshape(BT, C))
    W_qkv = np.asarray(W_qkv, dtype=np.float32)
    W_proj = np.ascontiguousarray(np.asarray(W_proj, dtype=np.float32))
    xr = x.reshape(B, NCORES, CHK, C)
    in_maps = []
    for c in range(NCORES):
        cols = slice(QKC * c, QKC * (c + 1))
        w_c = np.ascontiguousarray(
            np.concatenate(
                [W_qkv[:, cols], W_qkv[:, C:][:, cols], W_qkv[:, 2 * C :][:, cols]],
                axis=1,
            )
        )
        x_c = np.ascontiguousarray(xr[:, c].reshape(B * CHK, C))
        in_maps.append({"xown": x_c, "wqkv": w_c, "wproj": W_proj})
    return in_maps


def run(in_maps, trace=False, **kwargs):
    return run_bass_kernel_spmd(
        _get_nc(), in_maps, core_ids=list(range(NCORES)), trace=trace, **kwargs
    )


def gather(res) -> np.ndarray:
    """Un-stripe: core c's out rows are [b*CHK, (b+1)*CHK) = batch b tokens
    [b*T + c*CHK, b*T + (c+1)*CHK)."""
    outs = np.stack([res.results[c]["out"] for c in range(NCORES)])  # [8, TPS, C]
    full = outs.reshape(NCORES, B, CHK, C).transpose(1, 0, 2, 3).reshape(B, T, C)
    return np.ascontiguousarray(full)


def kernel(x, W_qkv, W_proj):
    res = run(shard_inputs(x, W_qkv, W_proj), trace=False)
    return gather(res).astype(np.float32)


# revision 11
# speedup vs baseline: 2.0974x; 1.1693x over previous
"""Causal self-attention (B=4, T=2048, C=1024, 16 heads) on 8 trn2 NeuronCores.

Sharding: tensor-parallel over heads (2 heads/core) for QKV + attention.
The x-transpose (token-major -> channel-major), identical work on every
core, is itself sharded: each core PE-transposes only its 1/8 of the
tokens (fed as the separate `xown` input; cores never load full x) and
a per-batch AllGather shares the bf16 xT through DRAM.  After
attention, a per-batch AllToAll reshards from head-split to
token-split for the output projection.  Output tokens are striped:
core c owns, for every batch b, tokens [b*2048+c*256, b*2048+(c+1)*256);
the host gather interleaves them back.

All matmul operands are bf16 (full-rate PE, fp32 PSUM accumulation);
simulated end-to-end rel err ~3.4e-3 vs the 2e-2 gate.

Per-core pipeline (identical program on all cores; only the fed W_qkv
column-slice and xown token-slice differ):
  upfront: per batch, transpose own 256 tokens (16 PE transposes) and
  trigger AllGather #b -> xT_full[b] [8 src x 1024ch x 256tok] DRAM.
  per batch b, per 512-token window w:
    stage1: DMA xT window [128,8kc,512] from xT_full[b]; qT,kT
            [128ch x tok] and v [tok-major, PE-transposed] from
            x @ W_qkv_slice (evacuations cast PSUM f32 -> bf16).
    stage2: causal attention per head: S^T tiles [kt=128, q=512],
            exp on ACT -> bf16 pt (scores/8 ~ N(0,1), no max
            subtraction), 0/1 mask on diagonal blocks (DVE), AV
            accumulation with a ones-column in v so PSUM row 64
            carries softmax denominators; normalize via 1/d =
            exp(-ln d) on ACT + gpsimd partition_broadcast + DVE mul.
  after batch b: AllToAll #b (512KB/rank bf16); proj for batch b-1
  runs under batch b's compute; only A2A #3 + proj #3 expose a tail.

Engine assignment: PE: matmuls + shard-transposes.  ACT: exp, yu/ob
evac, ln/exp reciprocal.  DVE: casts, evacuations, diag masks,
normalize mul.  GpSimd: partition_broadcast + collective triggers.
"""

import os
import numpy as np

from concourse import bass, bacc, mybir, tile
from concourse.bass_utils import run_bass_kernel_spmd

F32 = mybir.dt.float32
BF16 = mybir.dt.bfloat16

B, T, C = 4, 2048, 1024
H, D = 16, 64
NCORES = 8
HPC = H // NCORES            # heads per core = 2
QKC = HPC * D                # per-core q/k/v channels = 128
BT = B * T                   # 8192 tokens total
TPS = BT // NCORES           # tokens per core after A2A = 1024
CHK = TPS // B               # tokens per (core, batch) chunk = 256
P = 128
TW = 512                     # q window
NW = T // TW                 # windows per batch = 4
NKT = T // P                 # kt tiles per batch = 16
KC = C // P                  # contraction chunks = 8

DIAG_FIRST = os.environ.get("KDIAG", "1") == "1"


def _causal_mask_01() -> np.ndarray:
    """mask[p, m, f] = 1.0 iff kt_local = 128*m + p <= f, for q windows of 512."""
    m = np.zeros((P, NW, TW), dtype=np.float32)
    p = np.arange(P)[:, None, None]
    mm = np.arange(NW)[None, :, None]
    f = np.arange(TW)[None, None, :]
    m[(P * mm + p) <= f] = 1.0
    return m


def build() -> bass.Bass:
    nc = bacc.Bacc(num_devices=NCORES, target_bir_lowering=False)

    xown_d = nc.dram_tensor("xown", [B * CHK, C], F32, kind="ExternalInput")
    xb0_d = nc.dram_tensor("xb0", [T, C], F32, kind="ExternalInput")
    wqkv_d = nc.dram_tensor("wqkv", [C, 3 * QKC], F32, kind="ExternalInput")
    wproj_d = nc.dram_tensor("wproj", [C, C], F32, kind="ExternalInput")
    out_d = nc.dram_tensor("out", [TPS, C], F32, kind="ExternalOutput")

    mask_d = nc.inline_tensor(_causal_mask_01(), name="mask01")
    ident_d = nc.inline_tensor(np.eye(P, dtype=np.float32), name="ident")

    with tile.TileContext(nc) as tc:
        from contextlib import ExitStack

        with ExitStack() as ctx:
            # ---- persistent pools ----
            wq_pool = ctx.enter_context(tc.tile_pool(name="wq", bufs=1))
            dram = ctx.enter_context(tc.tile_pool(name="dram", bufs=1, space="DRAM"))

            wqkv_sb = wq_pool.tile([P, KC, 3 * QKC], BF16, tag="wqkv")
            mask_sb = wq_pool.tile([P, NW, TW], BF16, tag="mask")
            identb = wq_pool.tile([P, P], BF16, tag="identb")
            wproj_sb = wq_pool.tile([P, KC, C], BF16, tag="wproj")
            with tc.tile_pool(name="stage", bufs=2) as stg:
                idst = stg.tile([P, P], F32, tag="idst", bufs=1)
                nc.sync.dma_start(out=idst[:], in_=ident_d[:, :])
                nc.vector.tensor_copy(identb[:], idst[:])
                wqkv_st = stg.tile([P, KC, 3 * QKC], F32, tag="st3", bufs=1)
                nc.sync.dma_start(
                    out=wqkv_st[:],
                    in_=wqkv_d[:, :].rearrange("(k p) n -> p k n", p=P),
                )
                nc.vector.tensor_copy(wqkv_sb[:], wqkv_st[:])
                mask_st = stg.tile([P, NW, TW], F32, tag="stm", bufs=1)
                nc.sync.dma_start(out=mask_st[:], in_=mask_d[:, :, :])
                nc.vector.tensor_copy(mask_sb[:], mask_st[:])
                for kc in range(KC):
                    wproj_st = stg.tile([P, C], F32, tag="wst", name="wproj_st")
                    nc.sync.dma_start(
                        out=wproj_st[:], in_=wproj_d[kc * P : (kc + 1) * P, :]
                    )
                    nc.vector.tensor_copy(wproj_sb[:, kc, :], wproj_st[:])

            y_send = [
                dram.tile([NCORES, QKC, CHK], BF16, tag=f"ys{b}", name=f"y_send{b}")
                for b in range(B)
            ]
            y_recv = [
                dram.tile([NCORES, QKC, CHK], BF16, tag=f"yr{b}", name=f"y_recv{b}")
                for b in range(B)
            ]
            xpart = [
                dram.tile([C, CHK], BF16, tag=f"xp{b}", name=f"xpart{b}")
                for b in range(B)
            ]
            xT_full = [
                dram.tile([NCORES, C, CHK], BF16, tag=f"xf{b}", name=f"xT_full{b}")
                for b in range(B)
            ]

            # ---- working pools ----
            xT_pool = ctx.enter_context(tc.tile_pool(name="xT", bufs=2))
            qkv_pool = ctx.enter_context(tc.tile_pool(name="qkv", bufs=2))
            ps1 = ctx.enter_context(tc.tile_pool(name="ps1", bufs=2, space="PSUM"))
            pss = ctx.enter_context(tc.tile_pool(name="pss", bufs=2, space="PSUM"))
            psy = ctx.enter_context(tc.tile_pool(name="psy", bufs=2, space="PSUM"))
            pso = ctx.enter_context(tc.tile_pool(name="pso", bufs=2, space="PSUM"))
            pt_pool = ctx.enter_context(tc.tile_pool(name="pt", bufs=6))
            nrm_pool = ctx.enter_context(tc.tile_pool(name="nrm", bufs=2))
            yt_pool = ctx.enter_context(tc.tile_pool(name="yt", bufs=2))
            yr_pool = ctx.enter_context(tc.tile_pool(name="yr", bufs=2))
            ob_pool = ctx.enter_context(tc.tile_pool(name="ob", bufs=2))

            def allgather(b):
                nc.gpsimd.collective_compute(
                    "AllGather",
                    mybir.AluOpType.bypass,
                    replica_groups=[list(range(NCORES))],
                    ins=[xpart[b].opt()],
                    outs=[xT_full[b].opt()],
                )

            # ---- upfront: transpose own 256-token slice of batches 1..3
            # (batch 0 is transposed locally inside its windows: no AG wait)
            for b in range(1, B):
                xTp = xT_pool.tile([P, KC, CHK], BF16, tag="xTp", name="xTp")
                for s in range(CHK // P):
                    xn = xT_pool.tile([P, C], F32, tag="xn", name="xn", bufs=4)
                    nc.sync.dma_start(
                        out=xn[:],
                        in_=xown_d[b * CHK + s * P : b * CHK + (s + 1) * P, :],
                    )
                    xnb = xT_pool.tile([P, C], BF16, tag="xnb", name="xnb", bufs=4)
                    nc.vector.tensor_copy(xnb[:], xn[:])
                    for kcp in range(KC // 2):
                        ps_t = ps1.tile([P, 2, P], BF16, tag="ps1", name="ps_t")
                        for k2 in range(2):
                            kc = 2 * kcp + k2
                            nc.tensor.transpose(
                                ps_t[:, k2, :],
                                xnb[:, kc * P : (kc + 1) * P],
                                identb[:],
                            )
                        nc.vector.tensor_copy(
                            xTp[:, 2 * kcp : 2 * kcp + 2, s * P : (s + 1) * P],
                            ps_t[:],
                        )
                nc.sync.dma_start(
                    out=xpart[b][:, :].rearrange("(k p) t -> p k t", p=P),
                    in_=xTp[:],
                )
                allgather(b)

            def stage3(b):
                yr = yr_pool.tile([P, NCORES, CHK], BF16, tag="yr")
                nc.sync.dma_start(
                    out=yr[:], in_=y_recv[b][:, :, :].rearrange("k p t -> p k t")
                )
                for sub in range(CHK // P):
                    for half in range(C // TW):
                        ps_o = pso.tile([P, TW], F32, tag="ps_o")
                        for kc in range(KC):
                            nc.tensor.matmul(
                                ps_o[:],
                                lhsT=yr[:, kc, sub * P : (sub + 1) * P],
                                rhs=wproj_sb[:, kc, half * TW : (half + 1) * TW],
                                start=(kc == 0),
                                stop=(kc == KC - 1),
                            )
                        ob = ob_pool.tile([P, TW], F32, tag="ob")
                        nc.vector.tensor_copy(ob[:], ps_o[:])
                        nc.sync.dma_start(
                            out=out_d[
                                b * CHK + sub * P : b * CHK + (sub + 1) * P,
                                half * TW : (half + 1) * TW,
                            ],
                            in_=ob[:],
                        )

            for b in range(B):
                qT_b = qkv_pool.tile([P, T], BF16, tag="qT")
                kT_b = qkv_pool.tile([P, T], BF16, tag="kT")
                v_b = qkv_pool.tile([P, NKT, HPC, D + 1], BF16, tag="v")
                # ones column for softmax denominators
                nc.vector.memset(v_b[:, :, :, D : D + 1], 1.0)

                for w in range(NW):
                    # ---- stage 1: qT, kT, v for window w ----
                    xT = xT_pool.tile([P, KC, TW], BF16, tag="xT")
                    if b == 0:
                        xnbs = []
                        for s in range(TW // P):
                            xn = xT_pool.tile(
                                [P, C], F32, tag="xn", name="xn", bufs=4
                            )
                            nc.sync.dma_start(
                                out=xn[:],
                                in_=xb0_d[
                                    w * TW + s * P : w * TW + (s + 1) * P, :
                                ],
                            )
                            xnb = xT_pool.tile(
                                [P, C], BF16, tag="xnb", name="xnb", bufs=4
                            )
                            nc.vector.tensor_copy(xnb[:], xn[:])
                            xnbs.append(xnb)
                        for kc in range(KC):
                            ps_t = ps1.tile([P, TW], BF16, tag="ps1", name="ps_t")
                            for s in range(TW // P):
                                nc.tensor.transpose(
                                    ps_t[:, s * P : (s + 1) * P],
                                    xnbs[s][:, kc * P : (kc + 1) * P],
                                    identb[:],
                                )
                            nc.vector.tensor_copy(xT[:, kc, :], ps_t[:])
                    else:
                        for s in range(TW // CHK):
                            nc.sync.dma_start(
                                out=xT[:, :, s * CHK : (s + 1) * CHK],
                                in_=xT_full[b][2 * w + s, :, :].rearrange(
                                    "(k p) t -> p k t", p=P
                                ),
                            )
                    for which, dst in ((0, qT_b), (1, kT_b)):
                        ps = ps1.tile([P, TW], F32, tag="ps1")
                        for kc in range(KC):
                            nc.tensor.matmul(
                                ps[:],
                                lhsT=wqkv_sb[:, kc, which * QKC : (which + 1) * QKC],
                                rhs=xT[:, kc, :],
                                start=(kc == 0),
                                stop=(kc == KC - 1),
                            )
                        nc.vector.tensor_copy(dst[:, w * TW : (w + 1) * TW], ps[:])
                    ps_vT = ps1.tile([P, TW], F32, tag="ps1", name="ps_vT")
                    for kc in range(KC):
                        nc.tensor.matmul(
                            ps_vT[:],
                            lhsT=wqkv_sb[:, kc, 2 * QKC : 3 * QKC],
                            rhs=xT[:, kc, :],
                            start=(kc == 0),
                            stop=(kc == KC - 1),
                        )
                    vT_sb = xT_pool.tile([P, TW], BF16, tag="vT", name="vT_sb")
                    nc.vector.tensor_copy(vT_sb[:], ps_vT[:])
                    ps_v = ps1.tile([P, TW], BF16, tag="ps1", name="ps_v")
                    for s in range(TW // P):
                        nc.tensor.transpose(
                            ps_v[:, s * P : (s + 1) * P],
                            vT_sb[:, s * P : (s + 1) * P],
                            identb[:],
                        )
                    jt0 = w * (TW // P)
                    nc.vector.tensor_copy(
                        v_b[:, jt0 : jt0 + TW // P, :, 0:D],
                        ps_v[:].rearrange("p (s h d) -> p s h d", s=TW // P, h=HPC),
                    )

                    # ---- stage 2: attention for window w ----
                    for h in range(HPC):
                        qT_h = qT_b[h * D : (h + 1) * D, :]
                        kT_h = kT_b[h * D : (h + 1) * D, :]
                        nkt = (w + 1) * (TW // P)
                        ps_y = psy.tile([D + 1, TW], F32, tag="ps_y")
                        jks = list(range(nkt))
                        if DIAG_FIRST:
                            jks = jks[w * (TW // P) :] + jks[: w * (TW // P)]
                        for ji, jk in enumerate(jks):
                            ps_s = pss.tile([P, TW], F32, tag="ps_s")
                            nc.tensor.matmul(
                                ps_s[:],
                                lhsT=kT_h[:, jk * P : (jk + 1) * P],
                                rhs=qT_h[:, w * TW : (w + 1) * TW],
                                start=True,
                                stop=True,
                            )
                            pt = pt_pool.tile([P, TW], BF16, tag="pt")
                            nc.scalar.activation(
                                pt[:],
                                ps_s[:],
                                mybir.ActivationFunctionType.Exp,
                                scale=1.0 / np.sqrt(D),
                            )
                            m = jk - w * (TW // P)
                            if m >= 0:
                                nc.vector.tensor_mul(pt[:], pt[:], mask_sb[:, m, :])
                            nc.tensor.matmul(
                                ps_y[:],
                                lhsT=v_b[:, jk, h, :],
                                rhs=pt[:],
                                start=(ji == 0),
                                stop=(ji == nkt - 1),
                            )
                        yu = yt_pool.tile([D + 1, TW], F32, tag="yu", bufs=4)
                        nc.vector.tensor_copy(yu[:], ps_y[:])
                        # 1/d = exp(-ln d) on ACT (vector.reciprocal is ~3.3us)
                        lnd = nrm_pool.tile([1, TW], F32, tag="lnd")
                        nc.scalar.activation(
                            lnd[:], yu[D : D + 1, :], mybir.ActivationFunctionType.Ln
                        )
                        recip = nrm_pool.tile([1, TW], F32, tag="recip")
                        nc.scalar.activation(
                            recip[:],
                            lnd[:],
                            mybir.ActivationFunctionType.Exp,
                            scale=-1.0,
                        )
                        bc = nrm_pool.tile([D, TW], F32, tag="bc")
                        nc.gpsimd.partition_broadcast(bc[:], recip[:])
                        yt = yt_pool.tile([D, TW], BF16, tag="yt")
                        nc.vector.tensor_mul(yt[:], yu[0:D, :], bc[:])
                        for half in range(TW // CHK):
                            s = (TW // CHK) * w + half
                            nc.sync.dma_start(
                                out=y_send[b][s, h * D : (h + 1) * D, :],
                                in_=yt[:, half * CHK : (half + 1) * CHK],
                            )

                # ---- A2A for batch b; AG b+2; proj for batch b-1 ----
                nc.gpsimd.collective_compute(
                    "AllToAll",
                    mybir.AluOpType.bypass,
                    replica_groups=[list(range(NCORES))],
                    ins=[y_send[b].opt()],
                    outs=[y_recv[b].opt()],
                )
                if b > 0:
                    stage3(b - 1)
            stage3(B - 1)

    nc.finalize()
    return nc


_NC_CACHE: dict = {}


def _get_nc() -> bass.Bass:
    if "nc" not in _NC_CACHE:
        _NC_CACHE["nc"] = build()
    return _NC_CACHE["nc"]


def shard_inputs(x, W_qkv, W_proj):
    x = np.ascontiguousarray(np.asarray(x, dtype=np.float32).reshape(BT, C))
    W_qkv = np.asarray(W_qkv, dtype=np.float32)
    W_proj = np.ascontiguousarray(np.asarray(W_proj, dtype=np.float32))
    xr = x.reshape(B, NCORES, CHK, C)
    in_maps = []
    for c in range(NCORES):
        cols = slice(QKC * c, QKC * (c + 1))
        w_c = np.ascontiguousarray(
            np.concatenate(
                [W_qkv[:, cols], W_qkv[:, C:][:, cols], W_qkv[:, 2 * C :][:, cols]],
                axis=1,
            )
        )
        x_c = np.ascontiguousarray(xr[:, c].reshape(B * CHK, C))
        in_maps.append(
            {"xown": x_c, "xb0": x[:T], "wqkv": w_c, "wproj": W_proj}
        )
    return in_maps


def run(in_maps, trace=False, **kwargs):
    return run_bass_kernel_spmd(
        _get_nc(), in_maps, core_ids=list(range(NCORES)), trace=trace, **kwargs
    )


def gather(res) -> np.ndarray:
    """Un-stripe: core c's out rows are [b*CHK, (b+1)*CHK) = batch b tokens
    [b*T + c*CHK, b*T + (c+1)*CHK)."""
    outs = np.stack([res.results[c]["out"] for c in range(NCORES)])  # [8, TPS, C]
    full = outs.reshape(NCORES, B, CHK, C).transpose(1, 0, 2, 3).reshape(B, T, C)
    return np.ascontiguousarray(full)


def kernel(x, W_qkv, W_proj):
    res = run(shard_inputs(x, W_qkv, W_proj), trace=False)
    return gather(res).astype(np.float32)


# revision 12
# speedup vs baseline: 2.2938x; 1.0937x over previous
"""Causal self-attention (B=4, T=2048, C=1024, 16 heads) on 8 trn2 NeuronCores.

Sharding: tensor-parallel over heads (2 heads/core) for QKV + attention.
The x-transpose (token-major -> channel-major), identical work on every
core, is itself sharded: each core PE-transposes only its 1/8 of the
tokens (fed as the separate `xown` input; cores never load full x) and
a per-batch AllGather shares the bf16 xT through DRAM.  After
attention, a per-batch AllToAll reshards from head-split to
token-split for the output projection.  Output tokens are striped:
core c owns, for every batch b, tokens [b*2048+c*256, b*2048+(c+1)*256);
the host gather interleaves them back.

All matmul operands are bf16 (full-rate PE, fp32 PSUM accumulation);
simulated end-to-end rel err ~3.4e-3 vs the 2e-2 gate.

Per-core pipeline (identical program on all cores; only the fed W_qkv
column-slice and xown token-slice differ):
  upfront: per batch, transpose own 256 tokens (16 PE transposes) and
  trigger AllGather #b -> xT_full[b] [8 src x 1024ch x 256tok] DRAM.
  per batch b, per 512-token window w:
    stage1: DMA xT window [128,8kc,512] from xT_full[b]; qT,kT
            [128ch x tok] and v [tok-major, PE-transposed] from
            x @ W_qkv_slice (evacuations cast PSUM f32 -> bf16).
    stage2: causal attention per head: S^T tiles [kt=128, q=512],
            exp on ACT -> bf16 pt (scores/8 ~ N(0,1), no max
            subtraction), 0/1 mask on diagonal blocks (DVE), AV
            accumulation with a ones-column in v so PSUM row 64
            carries softmax denominators; normalize via 1/d =
            exp(-ln d) on ACT + gpsimd partition_broadcast + DVE mul.
  after batch b: AllToAll #b (512KB/rank bf16); proj for batch b-1
  runs under batch b's compute; only A2A #3 + proj #3 expose a tail.

Engine assignment: PE: matmuls + shard-transposes.  ACT: exp, yu/ob
evac, ln/exp reciprocal.  DVE: casts, evacuations, diag masks,
normalize mul.  GpSimd: partition_broadcast + collective triggers.
"""

import os
import numpy as np

from concourse import bass, bacc, mybir, tile
from concourse.bass_utils import run_bass_kernel_spmd

F32 = mybir.dt.float32
BF16 = mybir.dt.bfloat16

B, T, C = 4, 2048, 1024
H, D = 16, 64
NCORES = 8
HPC = H // NCORES            # heads per core = 2
QKC = HPC * D                # per-core q/k/v channels = 128
BT = B * T                   # 8192 tokens total
TPS = BT // NCORES           # tokens per core after A2A = 1024
CHK = TPS // B               # tokens per (core, batch) chunk = 256
P = 128
TW = 512                     # q window
NW = T // TW                 # windows per batch = 4
NKT = T // P                 # kt tiles per batch = 16
KC = C // P                  # contraction chunks = 8

DIAG_FIRST = os.environ.get("KDIAG", "1") == "1"


def _causal_mask_01() -> np.ndarray:
    """mask[p, m, f] = 1.0 iff kt_local = 128*m + p <= f, for q windows of 512."""
    m = np.zeros((P, NW, TW), dtype=np.float32)
    p = np.arange(P)[:, None, None]
    mm = np.arange(NW)[None, :, None]
    f = np.arange(TW)[None, None, :]
    m[(P * mm + p) <= f] = 1.0
    return m


def build() -> bass.Bass:
    nc = bacc.Bacc(num_devices=NCORES, target_bir_lowering=False)

    xown_d = nc.dram_tensor("xown", [B * CHK, C], F32, kind="ExternalInput")
    xb0_d = nc.dram_tensor("xb0", [T, C], F32, kind="ExternalInput")
    wqkv_d = nc.dram_tensor("wqkv", [C, 3 * QKC], F32, kind="ExternalInput")
    wproj_d = nc.dram_tensor("wproj", [C, C], F32, kind="ExternalInput")
    out_d = nc.dram_tensor("out", [TPS, C], F32, kind="ExternalOutput")

    mask_d = nc.inline_tensor(_causal_mask_01(), name="mask01")
    ident_d = nc.inline_tensor(np.eye(P, dtype=np.float32), name="ident")

    with tile.TileContext(nc) as tc:
        from contextlib import ExitStack

        with ExitStack() as ctx:
            # ---- persistent pools ----
            wq_pool = ctx.enter_context(tc.tile_pool(name="wq", bufs=1))
            dram = ctx.enter_context(tc.tile_pool(name="dram", bufs=1, space="DRAM"))

            wqkv_sb = wq_pool.tile([P, KC, 3 * QKC], BF16, tag="wqkv")
            mask_sb = wq_pool.tile([P, NW, TW], BF16, tag="mask")
            identb = wq_pool.tile([P, P], BF16, tag="identb")
            wproj_sb = wq_pool.tile([P, KC, C], BF16, tag="wproj")
            with tc.tile_pool(name="stage0", bufs=1) as stg0:
                idst = stg0.tile([P, P], F32, tag="idst", bufs=1)
                nc.sync.dma_start(out=idst[:], in_=ident_d[:, :])
                nc.vector.tensor_copy(identb[:], idst[:])

            y_send = [
                dram.tile([NCORES, QKC, CHK], BF16, tag=f"ys{b}", name=f"y_send{b}")
                for b in range(B)
            ]
            y_recv = [
                dram.tile([NCORES, QKC, CHK], BF16, tag=f"yr{b}", name=f"y_recv{b}")
                for b in range(B)
            ]
            xpart = [
                dram.tile([C, CHK], BF16, tag=f"xp{b}", name=f"xpart{b}")
                for b in range(B)
            ]
            xT_full = [
                dram.tile([NCORES, C, CHK], BF16, tag=f"xf{b}", name=f"xT_full{b}")
                for b in range(B)
            ]

            # ---- working pools ----
            xT_pool = ctx.enter_context(tc.tile_pool(name="xT", bufs=2))
            qkv_pool = ctx.enter_context(tc.tile_pool(name="qkv", bufs=2))
            ps1 = ctx.enter_context(tc.tile_pool(name="ps1", bufs=2, space="PSUM"))
            pss = ctx.enter_context(tc.tile_pool(name="pss", bufs=3, space="PSUM"))
            psy = ctx.enter_context(tc.tile_pool(name="psy", bufs=2, space="PSUM"))
            pso = ctx.enter_context(tc.tile_pool(name="pso", bufs=1, space="PSUM"))
            pt_pool = ctx.enter_context(tc.tile_pool(name="pt", bufs=6))
            nrm_pool = ctx.enter_context(tc.tile_pool(name="nrm", bufs=2))
            yt_pool = ctx.enter_context(tc.tile_pool(name="yt", bufs=2))
            yr_pool = ctx.enter_context(tc.tile_pool(name="yr", bufs=2))
            ob_pool = ctx.enter_context(tc.tile_pool(name="ob", bufs=2))

            def allgather(b):
                nc.gpsimd.collective_compute(
                    "AllGather",
                    mybir.AluOpType.bypass,
                    replica_groups=[list(range(NCORES))],
                    ins=[xpart[b].opt()],
                    outs=[xT_full[b].opt()],
                )

            # ---- upfront: transpose own 256-token slice of batches 1..3
            # (batch 0 is transposed locally inside its windows: no AG wait)
            for b in range(1, B):
                xTp = xT_pool.tile([P, KC, CHK], BF16, tag="xTp", name="xTp")
                for s in range(CHK // P):
                    xn = xT_pool.tile([P, C], F32, tag="xn", name="xn", bufs=4)
                    nc.sync.dma_start(
                        out=xn[:],
                        in_=xown_d[b * CHK + s * P : b * CHK + (s + 1) * P, :],
                    )
                    xnb = xT_pool.tile([P, C], BF16, tag="xnb", name="xnb", bufs=4)
                    nc.vector.tensor_copy(xnb[:], xn[:])
                    for kcp in range(KC // 2):
                        ps_t = ps1.tile([P, 2, P], BF16, tag="ps1", name="ps_t")
                        for k2 in range(2):
                            kc = 2 * kcp + k2
                            nc.tensor.transpose(
                                ps_t[:, k2, :],
                                xnb[:, kc * P : (kc + 1) * P],
                                identb[:],
                            )
                        nc.vector.tensor_copy(
                            xTp[:, 2 * kcp : 2 * kcp + 2, s * P : (s + 1) * P],
                            ps_t[:],
                        )
                nc.gpsimd.dma_start(
                    out=xpart[b][:, :].rearrange("(k p) t -> p k t", p=P),
                    in_=xTp[:],
                )
                allgather(b)

            with tc.tile_pool(name="stage", bufs=2) as stg:
                wqkv_st = stg.tile([P, KC, 3 * QKC], F32, tag="st3", bufs=1)
                nc.sync.dma_start(
                    out=wqkv_st[:],
                    in_=wqkv_d[:, :].rearrange("(k p) n -> p k n", p=P),
                )
                nc.vector.tensor_copy(wqkv_sb[:], wqkv_st[:])
                mask_st = stg.tile([P, NW, TW], F32, tag="stm", bufs=1)
                nc.sync.dma_start(out=mask_st[:], in_=mask_d[:, :, :])
                nc.vector.tensor_copy(mask_sb[:], mask_st[:])
                for kc in range(KC):
                    wproj_st = stg.tile([P, C], F32, tag="wst", name="wproj_st")
                    nc.sync.dma_start(
                        out=wproj_st[:], in_=wproj_d[kc * P : (kc + 1) * P, :]
                    )
                    nc.vector.tensor_copy(wproj_sb[:, kc, :], wproj_st[:])

            def stage3(b):
                yr = yr_pool.tile([P, NCORES, CHK], BF16, tag="yr")
                nc.sync.dma_start(
                    out=yr[:], in_=y_recv[b][:, :, :].rearrange("k p t -> p k t")
                )
                for sub in range(CHK // P):
                    for half in range(C // TW):
                        ps_o = pso.tile([P, TW], F32, tag="ps_o")
                        for kc in range(KC):
                            nc.tensor.matmul(
                                ps_o[:],
                                lhsT=yr[:, kc, sub * P : (sub + 1) * P],
                                rhs=wproj_sb[:, kc, half * TW : (half + 1) * TW],
                                start=(kc == 0),
                                stop=(kc == KC - 1),
                            )
                        ob = ob_pool.tile([P, TW], F32, tag="ob")
                        nc.vector.tensor_copy(ob[:], ps_o[:])
                        nc.gpsimd.dma_start(
                            out=out_d[
                                b * CHK + sub * P : b * CHK + (sub + 1) * P,
                                half * TW : (half + 1) * TW,
                            ],
                            in_=ob[:],
                        )

            for b in range(B):
                qT_b = qkv_pool.tile([P, T], BF16, tag="qT")
                kT_b = qkv_pool.tile([P, T], BF16, tag="kT")
                v_b = qkv_pool.tile([P, NKT, HPC, D + 1], BF16, tag="v")
                # ones column for softmax denominators
                nc.vector.memset(v_b[:, :, :, D : D + 1], 1.0)

                for w in range(NW):
                    # ---- stage 1: qT, kT, v for window w ----
                    xT = xT_pool.tile([P, KC, TW], BF16, tag="xT")
                    if b == 0:
                        xnbs = []
                        for s in range(TW // P):
                            xn = xT_pool.tile(
                                [P, C], F32, tag="xn", name="xn", bufs=4
                            )
                            nc.sync.dma_start(
                                out=xn[:],
                                in_=xb0_d[
                                    w * TW + s * P : w * TW + (s + 1) * P, :
                                ],
                            )
                            xnb = xT_pool.tile(
                                [P, C], BF16, tag="xnb", name="xnb", bufs=4
                            )
                            nc.vector.tensor_copy(xnb[:], xn[:])
                            xnbs.append(xnb)
                        for kc in range(KC):
                            ps_t = ps1.tile([P, TW], BF16, tag="ps1", name="ps_t")
                            for s in range(TW // P):
                                nc.tensor.transpose(
                                    ps_t[:, s * P : (s + 1) * P],
                                    xnbs[s][:, kc * P : (kc + 1) * P],
                                    identb[:],
                                )
                            nc.vector.tensor_copy(xT[:, kc, :], ps_t[:])
                    else:
                        for s in range(TW // CHK):
                            nc.sync.dma_start(
                                out=xT[:, :, s * CHK : (s + 1) * CHK],
                                in_=xT_full[b][2 * w + s, :, :].rearrange(
                                    "(k p) t -> p k t", p=P
                                ),
                            )
                    for which, dst in ((0, qT_b), (1, kT_b)):
                        ps = ps1.tile([P, TW], F32, tag="ps1")
                        for kc in range(KC):
                            nc.tensor.matmul(
                                ps[:],
                                lhsT=wqkv_sb[:, kc, which * QKC : (which + 1) * QKC],
                                rhs=xT[:, kc, :],
                                start=(kc == 0),
                                stop=(kc == KC - 1),
                            )
                        nc.vector.tensor_copy(dst[:, w * TW : (w + 1) * TW], ps[:])
                    ps_vT = ps1.tile([P, TW], F32, tag="ps1", name="ps_vT")
                    for kc in range(KC):
                        nc.tensor.matmul(
                            ps_vT[:],
                            lhsT=wqkv_sb[:, kc, 2 * QKC : 3 * QKC],
                            rhs=xT[:, kc, :],
                            start=(kc == 0),
                            stop=(kc == KC - 1),
                        )
                    vT_sb = xT_pool.tile([P, TW], BF16, tag="vT", name="vT_sb")
                    nc.vector.tensor_copy(vT_sb[:], ps_vT[:])
                    ps_v = ps1.tile([P, TW], BF16, tag="ps1", name="ps_v")
                    for s in range(TW // P):
                        nc.tensor.transpose(
                            ps_v[:, s * P : (s + 1) * P],
                            vT_sb[:, s * P : (s + 1) * P],
                            identb[:],
                        )
                    jt0 = w * (TW // P)
                    nc.vector.tensor_copy(
                        v_b[:, jt0 : jt0 + TW // P, :, 0:D],
                        ps_v[:].rearrange("p (s h d) -> p s h d", s=TW // P, h=HPC),
                    )

                    # ---- stage 2: attention for window w ----
                    for h in range(HPC):
                        qT_h = qT_b[h * D : (h + 1) * D, :]
                        kT_h = kT_b[h * D : (h + 1) * D, :]
                        nkt = (w + 1) * (TW // P)
                        ps_y = psy.tile([D + 1, TW], F32, tag="ps_y")
                        jks = list(range(nkt))
                        if DIAG_FIRST:
                            jks = jks[w * (TW // P) :] + jks[: w * (TW // P)]
                        for ji, jk in enumerate(jks):
                            ps_s = pss.tile([P, TW], F32, tag="ps_s")
                            nc.tensor.matmul(
                                ps_s[:],
                                lhsT=kT_h[:, jk * P : (jk + 1) * P],
                                rhs=qT_h[:, w * TW : (w + 1) * TW],
                                start=True,
                                stop=True,
                            )
                            pt = pt_pool.tile([P, TW], BF16, tag="pt")
                            nc.scalar.activation(
                                pt[:],
                                ps_s[:],
                                mybir.ActivationFunctionType.Exp,
                                scale=1.0 / np.sqrt(D),
                            )
                            m = jk - w * (TW // P)
                            if m >= 0:
                                nc.vector.tensor_mul(pt[:], pt[:], mask_sb[:, m, :])
                            nc.tensor.matmul(
                                ps_y[:],
                                lhsT=v_b[:, jk, h, :],
                                rhs=pt[:],
                                start=(ji == 0),
                                stop=(ji == nkt - 1),
                            )
                        yu = yt_pool.tile([D + 1, TW], F32, tag="yu", bufs=4)
                        nc.vector.tensor_copy(yu[:], ps_y[:])
                        # 1/d = exp(-ln d) on ACT (vector.reciprocal is ~3.3us)
                        lnd = nrm_pool.tile([1, TW], F32, tag="lnd")
                        nc.scalar.activation(
                            lnd[:], yu[D : D + 1, :], mybir.ActivationFunctionType.Ln
                        )
                        recip = nrm_pool.tile([1, TW], F32, tag="recip")
                        nc.scalar.activation(
                            recip[:],
                            lnd[:],
                            mybir.ActivationFunctionType.Exp,
                            scale=-1.0,
                        )
                        bc = nrm_pool.tile([D, TW], F32, tag="bc")
                        nc.gpsimd.partition_broadcast(bc[:], recip[:])
                        yt = yt_pool.tile([D, TW], BF16, tag="yt")
                        nc.vector.tensor_mul(yt[:], yu[0:D, :], bc[:])
                        for half in range(TW // CHK):
                            s = (TW // CHK) * w + half
                            nc.gpsimd.dma_start(
                                out=y_send[b][s, h * D : (h + 1) * D, :],
                                in_=yt[:, half * CHK : (half + 1) * CHK],
                            )

                # ---- A2A for batch b; AG b+2; proj for batch b-1 ----
                nc.gpsimd.collective_compute(
                    "AllToAll",
                    mybir.AluOpType.bypass,
                    replica_groups=[list(range(NCORES))],
                    ins=[y_send[b].opt()],
                    outs=[y_recv[b].opt()],
                )
                if b > 0:
                    stage3(b - 1)
            stage3(B - 1)

    nc.finalize()
    return nc


_NC_CACHE: dict = {}


def _get_nc() -> bass.Bass:
    if "nc" not in _NC_CACHE:
        _NC_CACHE["nc"] = build()
    return _NC_CACHE["nc"]


def shard_inputs(x, W_qkv, W_proj):
    x = np.ascontiguousarray(np.asarray(x, dtype=np.float32).reshape(BT, C))
    W_qkv = np.asarray(W_qkv, dtype=np.float32)
    W_proj = np.ascontiguousarray(np.asarray(W_proj, dtype=np.float32))
    xr = x.reshape(B, NCORES, CHK, C)
    in_maps = []
    for c in range(NCORES):
        cols = slice(QKC * c, QKC * (c + 1))
        w_c = np.ascontiguousarray(
            np.concatenate(
                [W_qkv[:, cols], W_qkv[:, C:][:, cols], W_qkv[:, 2 * C :][:, cols]],
                axis=1,
            )
        )
        x_c = np.ascontiguousarray(xr[:, c].reshape(B * CHK, C))
        in_maps.append(
            {"xown": x_c, "xb0": x[:T], "wqkv": w_c, "wproj": W_proj}
        )
    return in_maps


def run(in_maps, trace=False, **kwargs):
    return run_bass_kernel_spmd(
        _get_nc(), in_maps, core_ids=list(range(NCORES)), trace=trace, **kwargs
    )


def gather(res) -> np.ndarray:
    """Un-stripe: core c's out rows are [b*CHK, (b+1)*CHK) = batch b tokens
    [b*T + c*CHK, b*T + (c+1)*CHK)."""
    outs = np.stack([res.results[c]["out"] for c in range(NCORES)])  # [8, TPS, C]
    full = outs.reshape(NCORES, B, CHK, C).transpose(1, 0, 2, 3).reshape(B, T, C)
    return np.ascontiguousarray(full)


def kernel(x, W_qkv, W_proj):
    res = run(shard_inputs(x, W_qkv, W_proj), trace=False)
    return gather(res).astype(np.float32)
